# revision 7
# baseline (speedup 1.0000x reference)
"""Trainium2 Bass kernel for nn_Encoder_14121852469955.

6-layer post-LN transformer encoder (D=1024, H=16, F=4096, S=2048, B=2),
distributed over 8 NeuronCores.

Sharding: token-data-parallel. Each core owns 512 token rows (256 from each
batch element, an S/8 slice). Everything except attention is per-token and
runs locally with replicated weights. For attention, each layer AllGathers
K^T and V (bf16) across the 8 cores so every core attends its queries over
the full sequence of both batch elements.

On-device layout: activations are kept TRANSPOSED, xT = [D(partitions), T],
so every projection is a natural PE matmul (out = lhsT.T @ rhs with
contraction on partitions). Attention computes transposed scores
S^T = K @ Q^T per head, exp via the scalar engine (max-subtraction-free
softmax), and context via (V' appended with a ones column) so the softmax
denominator falls out of the same matmul chain. Matmul operands are bf16
(fp32 PSUM accumulation); the residual stream stays fp32 in SBUF.

Host side (inside kernel()): embedding gather + sinusoidal positions,
weight reshape/cast to bf16, sharding, and the final unshard/transpose.
"""

import numpy as np
import ml_dtypes

import concourse.mybir as mybir
import concourse.tile as tile
from concourse import bacc
from concourse import bass_utils

BF16 = mybir.dt.bfloat16
F32 = mybir.dt.float32
F32R = mybir.dt.float32r
AF = mybir.ActivationFunctionType
ALU = mybir.AluOpType

V, D, H, L, F, S, B = 32000, 1024, 16, 6, 4096, 2048, 2
HD = D // H  # 64
NCORES = 8
TB = S // NCORES  # 256 tokens per batch element per core
TC = B * TB  # 512 token rows per core
DT = D // 128  # 8 d-tiles
FT = F // 128  # 32 f-tiles
HP = H // 2  # 8 head pairs
VW = H * (HD + 1)  # 1040: V with a ones column interleaved per head
EPS = 1e-5

# column offsets inside the per-layer "smalls" [128, 104] tile
_SM = {"bq": 0, "bk": 8, "bv": 16, "bo": 24, "b2": 32,
       "g1": 40, "be1": 48, "g2": 56, "be2": 64, "b1": 72}
_SMW = 104


def _build_nc(reps=1):
    nc = bacc.Bacc("TRN2", target_bir_lowering=False, debug=False,
                   num_devices=NCORES)

    def inp(name, shape, dt):
        return nc.dram_tensor(name, shape, dt, kind="ExternalInput").ap()

    xT32 = inp("xT32", [D, TC], F32)
    xTbf = inp("xTbf", [D, TC], BF16)
    wq = inp("wq", [L, DT, 128, 1024], BF16)
    wk = inp("wk", [L, DT, 128, 1024], BF16)
    wo = inp("wo", [L, DT, 128, 1024], BF16)
    wv = inp("wv", [L, DT, 128, 1024], BF16)
    w1 = inp("w1", [L, FT, 128, 1024], BF16)
    w2 = inp("w2", [L, DT, 128, 4096], BF16)
    smalls = inp("smalls", [L, 128, _SMW], F32)
    outT = nc.dram_tensor("outT", [D, TC], F32, kind="ExternalOutput").ap()

    rg = [list(range(NCORES))]

    import contextlib
    with tile.TileContext(nc) as tc, contextlib.ExitStack() as ctx:
        ctx.enter_context(nc.allow_low_precision(
            "f32r norm scalars are intentional (~2^-13 rounding)"))
        pools = {}
        for name, bufs, space in [
            ("const", 1, "SBUF"), ("smalls", 2, "SBUF"),
            ("xres", 10, "SBUF"), ("xb", 10, "SBUF"),
            ("rb", 3, "SBUF"), ("sq", 3, "SBUF"),
            ("t1", 2, "SBUF"), ("t2", 2, "SBUF"),
            ("rows", 8, "SBUF"),
            ("kc", 3, "SBUF"), ("vc", 3, "SBUF"),
            ("kT", 2, "SBUF"), ("exp", 17, "SBUF"), ("vs", 6, "SBUF"),
            ("recb", 2, "SBUF"), ("ctxp", 2, "SBUF"),
            ("ctxT", 9, "SBUF"), ("qT", 9, "SBUF"),
            ("wq", 3, "SBUF"), ("wk", 3, "SBUF"), ("wo", 3, "SBUF"),
            ("wv", 9, "SBUF"), ("w1", 4, "SBUF"), ("w2", 4, "SBUF"),
            ("h1", 33, "SBUF"),
            ("pss", 2, "PSUM"), ("psc", 3, "PSUM"), ("psm", 3, "PSUM"),
            ("dram", 2, "DRAM"),
        ]:
            pools[name] = ctx.enter_context(
                tc.tile_pool(name=name, bufs=bufs, space=space))

        def P(name, shape, dt):
            return pools[name].tile(shape, dt, tag=name, name=name)

        # constants
        ones128 = P("const", [128, 1], BF16)
        nc.vector.memset(ones128[:], 1.0 / D)
        ones1f = pools["const"].tile([1, 128], F32, tag="ones1f")
        nc.vector.memset(ones1f[:], 1.0)
        ones1r = pools["const"].tile([1, 128], F32R, tag="ones1r")
        nc.vector.tensor_copy(ones1r[:], ones1f[:])
        eps_t = pools["const"].tile([1, 1], F32, tag="eps_t")
        nc.vector.memset(eps_t[:], EPS)

        mm = nc.tensor.matmul

        def layernorm(rs, rbs, sm, gname, bname):
            """post-LN over d (partitions): rs fp32 [128,TC]x8, rbs bf16."""
            sqs = []
            for dt in range(DT):
                sqt = P("sq", [128, TC], BF16)
                nc.scalar.square(sqt[:], rbs[dt][:])
                sqs.append(sqt)
            mean_ps = P("psm", [1, TC], F32)
            for dt in range(DT):
                mm(mean_ps[:], ones128[:], rbs[dt][:],
                   start=(dt == 0), stop=(dt == DT - 1))
            ex2_ps = P("psm", [1, TC], F32)
            for dt in range(DT):
                mm(ex2_ps[:], ones128[:], sqs[dt][:],
                   start=(dt == 0), stop=(dt == DT - 1))
            meanS = pools["rows"].tile([1, TC], F32R, tag="rows")
            nc.vector.tensor_copy(meanS[:], mean_ps[:])
            msq = pools["rows"].tile([1, TC], F32R, tag="rows")
            nc.scalar.square(msq[:], meanS[:])
            var = pools["rows"].tile([1, TC], F32R, tag="rows")
            nc.vector.tensor_tensor(var[:], ex2_ps[:], msq[:], ALU.subtract)
            sstd = pools["rows"].tile([1, TC], F32R, tag="rows")
            nc.scalar.activation(sstd[:], var[:], AF.Sqrt, bias=eps_t[:])
            rstd = pools["rows"].tile([1, TC], F32R, tag="rows")
            nc.vector.reciprocal(rstd[:], sstd[:])
            meanB = P("psm", [128, TC], F32)
            mm(meanB[:], ones1r[:], meanS[:], start=True, stop=True)
            rstdB = P("psm", [128, TC], F32)
            mm(rstdB[:], ones1r[:], rstd[:], start=True, stop=True)
            xs, xbs = [], []
            for dt in range(DT):
                t1 = P("t1", [128, TC], F32)
                nc.vector.tensor_tensor(t1[:], rs[dt][:], meanB[:],
                                        ALU.subtract)
                t2 = P("t2", [128, TC], F32)
                nc.vector.tensor_tensor(t2[:], t1[:], rstdB[:], ALU.mult)
                xo = P("xres", [128, TC], F32)
                nc.vector.tensor_scalar(
                    out=xo[:], in0=t2[:],
                    scalar1=sm[:, _SM[gname] + dt: _SM[gname] + dt + 1],
                    scalar2=sm[:, _SM[bname] + dt: _SM[bname] + dt + 1],
                    op0=ALU.mult, op1=ALU.add)
                xob = P("xb", [128, TC], BF16)
                nc.vector.tensor_copy(xob[:], xo[:])
                xs.append(xo)
                xbs.append(xob)
            return xs, xbs

        def layer(l, x, xb):
            sm = P("smalls", [128, _SMW], F32)
            nc.sync.dma_start(sm[:], smalls[l])

            def col(name, i):
                c = _SM[name] + i
                return sm[:, c:c + 1]

            # ---- K^T (transposed keys), then AllGather ----
            kv_in_k = pools["dram"].tile([D * TC], BF16, tag="kv_in_k")
            kv_all_k = pools["dram"].tile([NCORES * D * TC], BF16,
                                          tag="kv_all_k",
                                          addr_space="Shared")
            kin = kv_in_k[:].rearrange("(d p c) -> d p c", d=DT, p=128)
            for dt in range(DT):
                wkt = P("wk", [128, 1024], BF16)
                nc.sync.dma_start(wkt[:], wk[l, dt])
                ps = P("psm", [128, TC], F32)
                for kt in range(DT):
                    mm(ps[:], wkt[:, 128 * kt:128 * (kt + 1)], xb[kt][:],
                       start=(kt == 0), stop=(kt == DT - 1))
                kc = P("kc", [128, TC], BF16)
                nc.scalar.activation(kc[:], ps[:], AF.Identity,
                                     bias=col("bk", dt))
                nc.sync.dma_start(kin[dt], kc[:])
            nc.gpsimd.collective_compute(
                "AllGather", ALU.bypass, replica_groups=rg,
                ins=[kv_in_k.opt()], outs=[kv_all_k.opt()])

            # ---- V (token-major, ones column per head), then AllGather ----
            kv_in_v = pools["dram"].tile([TC * VW], BF16, tag="kv_in_v")
            kv_all_v = pools["dram"].tile([NCORES * TC * VW], BF16,
                                          tag="kv_all_v",
                                          addr_space="Shared")
            wv_sb = []
            for kt in range(DT):
                wvt = P("wv", [128, 1024], BF16)
                nc.sync.dma_start(wvt[:], wv[l, kt])
                wv_sb.append(wvt)
            vin = kv_in_v[:].rearrange("(t p c) -> t p c", t=4, p=128)
            for tt in range(4):
                vc = P("vc", [128, VW], BF16)
                vcr = vc[:].rearrange("p (h c) -> p h c", c=HD + 1)
                nc.vector.memset(vcr[:, :, HD:HD + 1], 1.0)
                for half in range(2):
                    ps = P("psm", [128, 512], F32)
                    for kt in range(DT):
                        mm(ps[:], xb[kt][:, 128 * tt:128 * (tt + 1)],
                           wv_sb[kt][:, 512 * half:512 * (half + 1)],
                           start=(kt == 0), stop=(kt == DT - 1))
                    nc.vector.tensor_copy(
                        vcr[:, 8 * half:8 * (half + 1), 0:HD],
                        ps[:].rearrange("p (h c) -> p h c", c=HD))
                nc.sync.dma_start(vin[tt], vc[:])
            nc.gpsimd.collective_compute(
                "AllGather", ALU.bypass, replica_groups=rg,
                ins=[kv_in_v.opt()], outs=[kv_all_v.opt()])

            # ---- Q^T (stays local) ----
            qT = []
            for dt in range(DT):
                wqt = P("wq", [128, 1024], BF16)
                nc.sync.dma_start(wqt[:], wq[l, dt])
                ps = P("psm", [128, TC], F32)
                for kt in range(DT):
                    mm(ps[:], wqt[:, 128 * kt:128 * (kt + 1)], xb[kt][:],
                       start=(kt == 0), stop=(kt == DT - 1))
                qt = P("qT", [128, TC], BF16)
                nc.scalar.activation(qt[:], ps[:], AF.Identity,
                                     bias=col("bq", dt))
                qT.append(qt)

            # ---- attention ----
            kall = kv_all_k[:].rearrange("(s d p c) -> s d p c",
                                         s=NCORES, d=DT, p=128)
            vall = kv_all_v[:].rearrange("(s t c) -> s t c", s=NCORES, t=TC)
            ctxT = [P("ctxT", [128, TC], BF16) for _ in range(DT)]
            for b in range(B):
                for hp in range(HP):
                    kT = P("kT", [128, NCORES * TB], BF16)
                    src = kall[:, hp, :, TB * b:TB * (b + 1)].rearrange(
                        "s p c -> p s c")
                    nc.sync.dma_start(
                        kT[:].rearrange("p (s c) -> p s c", s=NCORES), src)
                    exps = {}
                    for hsub in range(2):
                        for s in range(NCORES):
                            ps = P("pss", [128, 2 * TB], F32)
                            for k2 in range(2):
                                mm(ps[:, TB * k2:TB * (k2 + 1)],
                                   kT[64 * hsub:64 * (hsub + 1),
                                      256 * s + 128 * k2:
                                      256 * s + 128 * (k2 + 1)],
                                   qT[hp][64 * hsub:64 * (hsub + 1),
                                          TB * b:TB * (b + 1)],
                                   start=True, stop=True)
                            ex = P("exp", [128, 2 * TB], BF16)
                            nc.scalar.activation(ex[:], ps[:], AF.Exp,
                                                 scale=1.0 / np.sqrt(HD))
                            exps[(hsub, s)] = ex
                    ctx_ps = [P("psc", [HD + 1, TB], F32) for _ in range(2)]
                    for kt in range(2 * NCORES):
                        s_, k2 = kt // 2, kt % 2
                        vs = P("vs", [128, 2 * (HD + 1)], BF16)
                        nc.sync.dma_start(
                            vs[:],
                            vall[s_, TB * b + 128 * k2:TB * b + 128 * (k2 + 1),
                                 130 * hp:130 * (hp + 1)])
                        for hsub in range(2):
                            mm(ctx_ps[hsub][:],
                               vs[:, 65 * hsub:65 * (hsub + 1)],
                               exps[(hsub, s_)][:, TB * k2:TB * (k2 + 1)],
                               start=(kt == 0), stop=(kt == 2 * NCORES - 1))
                    for hsub in range(2):
                        rec = pools["rows"].tile([1, TB], F32R, tag="rows")
                        nc.vector.reciprocal(rec[:],
                                             ctx_ps[hsub][HD:HD + 1, :])
                        rB = P("psc", [HD, TB], F32)
                        mm(rB[:], ones1r[:, 0:HD], rec[:],
                           start=True, stop=True)
                        rBs = P("recb", [HD, TB], F32)
                        nc.vector.tensor_copy(rBs[:], rB[:])
                        t = P("ctxp", [HD, TB], F32)
                        nc.vector.tensor_tensor(t[:], ctx_ps[hsub][0:HD, :],
                                                rBs[:], ALU.mult)
                        nc.vector.tensor_scalar(
                            out=ctxT[hp][64 * hsub:64 * (hsub + 1),
                                         TB * b:TB * (b + 1)],
                            in0=t[:],
                            scalar1=sm[64 * hsub:64 * (hsub + 1),
                                       _SM["bv"] + hp:_SM["bv"] + hp + 1],
                            scalar2=None, op0=ALU.add)

            # ---- Wo + residual, then LN1 ----
            rs, rbs = [], []
            for dt in range(DT):
                wot = P("wo", [128, 1024], BF16)
                nc.sync.dma_start(wot[:], wo[l, dt])
                ps = P("psm", [128, TC], F32)
                for kt in range(DT):
                    mm(ps[:], wot[:, 128 * kt:128 * (kt + 1)], ctxT[kt][:],
                       start=(kt == 0), stop=(kt == DT - 1))
                r = P("xres", [128, TC], F32)
                nc.vector.scalar_tensor_tensor(
                    r[:], ps[:], col("bo", dt), x[dt][:], ALU.add, ALU.add)
                rb = P("rb", [128, TC], BF16)
                nc.vector.tensor_copy(rb[:], r[:])
                rs.append(r)
                rbs.append(rb)
            x2, x2b = layernorm(rs, rbs, sm, "g1", "be1")

            # ---- FFN ----
            h1 = []
            for ft in range(FT):
                w1t = P("w1", [128, 1024], BF16)
                nc.sync.dma_start(w1t[:], w1[l, ft])
                ps = P("psm", [128, TC], F32)
                for kt in range(DT):
                    mm(ps[:], w1t[:, 128 * kt:128 * (kt + 1)], x2b[kt][:],
                       start=(kt == 0), stop=(kt == DT - 1))
                h1t = P("h1", [128, TC], BF16)
                nc.scalar.activation(h1t[:], ps[:], AF.Relu,
                                     bias=col("b1", ft))
                h1.append(h1t)
            rs, rbs = [], []
            for dt in range(DT):
                ps = P("psm", [128, TC], F32)
                for q4 in range(4):
                    w2t = P("w2", [128, 1024], BF16)
                    nc.sync.dma_start(
                        w2t[:], w2[l, dt][:, 1024 * q4:1024 * (q4 + 1)])
                    for k8 in range(8):
                        kt = 8 * q4 + k8
                        mm(ps[:], w2t[:, 128 * k8:128 * (k8 + 1)],
                           h1[kt][:], start=(kt == 0), stop=(kt == FT - 1))
                r = P("xres", [128, TC], F32)
                nc.vector.scalar_tensor_tensor(
                    r[:], ps[:], col("b2", dt), x2[dt][:], ALU.add, ALU.add)
                rb = P("rb", [128, TC], BF16)
                nc.vector.tensor_copy(rb[:], r[:])
                rs.append(r)
                rbs.append(rb)
            return layernorm(rs, rbs, sm, "g2", "be2")

        # load x, run layers, store
        x, xb = [], []
        xv = xT32.rearrange("(d p) c -> d p c", p=128)
        xbv = xTbf.rearrange("(d p) c -> d p c", p=128)
        for dt in range(DT):
            xt = P("xres", [128, TC], F32)
            nc.sync.dma_start(xt[:], xv[dt])
            x.append(xt)
            xbt = P("xb", [128, TC], BF16)
            nc.sync.dma_start(xbt[:], xbv[dt])
            xb.append(xbt)
        for _ in range(reps):
            for l in range(L):
                x, xb = layer(l, x, xb)
        ov = outT.rearrange("(d p) c -> d p c", p=128)
        for dt in range(DT):
            nc.sync.dma_start(ov[dt], x[dt][:])

    nc.compile()
    return nc


def _pos_encoding():
    pos = np.arange(S, dtype=np.float32)[:, None]
    div = np.exp(np.arange(0, D, 2, dtype=np.float32)
                 * np.float32(-np.log(10000.0) / D))
    pe = np.zeros((S, D), dtype=np.float32)
    pe[:, 0::2] = np.sin(pos * div)
    pe[:, 1::2] = np.cos(pos * div)
    return pe


def _prep_inputs(tokens, mask, emb, Wq, bq, Wk, bk, Wv, bv, Wo, bo,
                 W1, b1, W2, b2, g1, be1, g2, be2):
    bf = ml_dtypes.bfloat16
    f32 = np.float32

    def np32(a):
        return np.ascontiguousarray(np.asarray(a, dtype=f32))

    x = np32(emb)[np.asarray(tokens)] + _pos_encoding()[None]  # [B, S, D]

    def colmajor(w, n_out_tiles):
        # w [L, K, N] -> [L, n_out_tiles, 128, K] with cols kt-major
        Lk, K, N = w.shape
        r = w.reshape(Lk, K // 128, 128, n_out_tiles, N // n_out_tiles)
        return np.ascontiguousarray(
            r.transpose(0, 3, 2, 1, 4).reshape(Lk, n_out_tiles, 128, -1)
        ).astype(bf)

    shared = {
        "wq": colmajor(np32(Wq), DT),
        "wk": colmajor(np32(Wk), DT),
        "wo": colmajor(np32(Wo), DT),
        "wv": np.ascontiguousarray(
            np32(Wv).reshape(L, DT, 128, D)).astype(bf),
        "w1": colmajor(np32(W1), FT),
        "w2": colmajor(np32(W2), DT),
    }
    smalls = np.zeros((L, 128, _SMW), dtype=f32)
    for name, vec, nt in [("bq", bq, DT), ("bk", bk, DT), ("bv", bv, DT),
                          ("bo", bo, DT), ("b2", b2, DT), ("g1", g1, DT),
                          ("be1", be1, DT), ("g2", g2, DT), ("be2", be2, DT),
                          ("b1", b1, FT)]:
        v = np32(vec).reshape(L, nt, 128)
        smalls[:, :, _SM[name]:_SM[name] + nt] = v.transpose(0, 2, 1)
    shared["smalls"] = smalls

    in_maps = []
    for c in range(NCORES):
        xc = np.concatenate(
            [x[0, TB * c:TB * (c + 1)], x[1, TB * c:TB * (c + 1)]], axis=0)
        xT = np.ascontiguousarray(xc.T.astype(f32))
        m = dict(shared)
        m["xT32"] = xT
        m["xTbf"] = xT.astype(bf)
        in_maps.append(m)
    return in_maps


_NC_CACHE = None


def _get_nc():
    global _NC_CACHE
    if _NC_CACHE is None:
        _NC_CACHE = _build_nc()
    return _NC_CACHE


def _unshard(results):
    out = np.empty((B, S, D), dtype=np.float32)
    for c in range(NCORES):
        xc = results[c]["outT"].T  # [TC, D]
        out[0, TB * c:TB * (c + 1)] = xc[:TB]
        out[1, TB * c:TB * (c + 1)] = xc[TB:]
    return out


def kernel(**inputs) -> np.ndarray:
    in_maps = _prep_inputs(**inputs)
    nc = _get_nc()
    res = bass_utils.run_bass_kernel_spmd(
        nc, in_maps, core_ids=list(range(NCORES)))
    return _unshard(res.results)


# revision 22
# speedup vs baseline: 25.6230x; 25.6230x over previous
"""Trainium2 Bass kernel for nn_Encoder_14121852469955.

6-layer post-LN transformer encoder (D=1024, H=16, F=4096, S=2048, B=2),
distributed over 8 NeuronCores.

Sharding: token-data-parallel. Each core owns 512 token rows (256 from each
batch element, an S/8 slice). Everything except attention is per-token and
runs locally with replicated weights. For attention, each layer AllGathers
K^T and V (bf16) across the 8 cores so every core attends its queries over
the full sequence of both batch elements.

On-device layout: activations are kept TRANSPOSED, xT = [D(partitions), T],
so every projection is a natural PE matmul (out = lhsT.T @ rhs with
contraction on partitions). Attention computes transposed scores
S^T = K @ Q^T per head, exp via the scalar engine (max-subtraction-free
softmax), and context via (V' appended with a ones column) so the softmax
denominator falls out of the same matmul chain. Matmul operands are bf16
(fp32 PSUM accumulation); the residual stream stays fp32 in SBUF.

Host side (inside kernel()): embedding gather + sinusoidal positions,
weight reshape/cast to bf16, sharding, and the final unshard/transpose.
"""

import numpy as np
import ml_dtypes

import concourse.mybir as mybir
import concourse.tile as tile
from concourse import bacc
from concourse import bass_utils

BF16 = mybir.dt.bfloat16
F32 = mybir.dt.float32
F32R = mybir.dt.float32r
AF = mybir.ActivationFunctionType
ALU = mybir.AluOpType

V, D, H, L, F, S, B = 32000, 1024, 16, 6, 4096, 2048, 2
HD = D // H  # 64
NCORES = 8
TB = S // NCORES  # 256 tokens per batch element per core
TC = B * TB  # 512 token rows per core
DT = D // 128  # 8 d-tiles
FT = F // 128  # 32 f-tiles
HP = H // 2  # 8 head pairs
VW = H * (HD + 1)  # 1040: V with a ones column interleaved per head
EPS = 1e-5

# column offsets inside the per-layer "smalls" [128, 104] tile
_SM = {"bq": 0, "bk": 8, "bv": 16, "bo": 24, "b2": 32,
       "g1": 40, "be1": 48, "g2": 56, "be2": 64, "b1": 72}
_SMW = 104


def _build_nc(reps=1, variant=()):
    """variant: experiment flags for timing A/B (results become garbage):
    "no_ag" (replace AllGathers with a local copy), "no_exp" (DVE copy
    instead of scalar-engine Exp), "local_kv" (attention reads local SBUF
    tiles instead of DMAing gathered K/V)."""
    variant = set(variant)
    nc = bacc.Bacc("TRN2", target_bir_lowering=False, debug=False,
                   num_devices=NCORES)

    def inp(name, shape, dt):
        return nc.dram_tensor(name, shape, dt, kind="ExternalInput").ap()

    xT32 = inp("xT32", [D, TC], F32)
    xTbf = inp("xTbf", [D, TC], BF16)
    wq = inp("wq", [L, DT, 128, 1024], BF16)
    wk = inp("wk", [L, DT, 128, 1024], BF16)
    wo = inp("wo", [L, DT, 128, 1024], BF16)
    wv = inp("wv", [L, DT, 128, 1024], BF16)
    w1 = inp("w1", [L, FT, 128, 1024], BF16)
    w2 = inp("w2", [L, DT, 128, 4096], BF16)
    smalls = inp("smalls", [L, 128, _SMW], F32)
    outT = nc.dram_tensor("outT", [D, TC], F32, kind="ExternalOutput").ap()

    rg = [list(range(NCORES))]

    import contextlib
    with tile.TileContext(nc) as tc, contextlib.ExitStack() as ctx:
        ctx.enter_context(nc.allow_low_precision(
            "f32r norm scalars are intentional (~2^-13 rounding)"))
        pools = {}
        for name, bufs, space in [
            ("const", 1, "SBUF"), ("smalls", 2, "SBUF"),
            ("xres", 10, "SBUF"), ("xb", 10, "SBUF"),
            ("rb", 2, "SBUF"), ("sq", 2, "SBUF"),
            ("t1", 2, "SBUF"), ("t2", 2, "SBUF"),
            ("rows", 8, "SBUF"),
            ("kc", 9 if "local_kv" in variant else 3, "SBUF"),
            ("vc", 6 if "local_kv" in variant else 3, "SBUF"),
            ("kT", 2, "SBUF"), ("exp", 17, "SBUF"), ("vs", 3, "SBUF"),
            ("recb", 2, "SBUF"), ("ctxp", 2, "SBUF"),
            ("ctxT", 9, "SBUF"), ("qT", 9, "SBUF"),
            ("wq", 2, "SBUF"), ("wk", 2, "SBUF"), ("wo", 2, "SBUF"),
            ("wv", 9, "SBUF"), ("w1", 4, "SBUF"), ("w2", 4, "SBUF"),
            ("h1", 33, "SBUF"),
            ("pss", 2, "PSUM"), ("psc", 3, "PSUM"), ("psm", 3, "PSUM"),
            ("dram", 2, "DRAM"),
        ]:
            pools[name] = ctx.enter_context(
                tc.tile_pool(name=name, bufs=bufs, space=space))

        def P(name, shape, dt):
            return pools[name].tile(shape, dt, tag=name, name=name)

        # constants
        ones128 = P("const", [128, 1], BF16)
        nc.vector.memset(ones128[:], 1.0 / D)
        ones1f = pools["const"].tile([1, 128], F32, tag="ones1f")
        nc.vector.memset(ones1f[:], 1.0)
        ones1r = pools["const"].tile([1, 128], F32R, tag="ones1r")
        nc.vector.tensor_copy(ones1r[:], ones1f[:])
        eps_t = pools["const"].tile([1, 1], F32, tag="eps_t")
        nc.vector.memset(eps_t[:], EPS)

        mm = nc.tensor.matmul

        def layernorm(rs, rbs, sm, gname, bname):
            """post-LN over d (partitions): rs fp32 [128,TC]x8, rbs bf16."""
            sqs = []
            for dt in range(DT):
                sqt = P("sq", [128, TC], BF16)
                nc.scalar.square(sqt[:], rbs[dt][:])
                sqs.append(sqt)
            mean_ps = P("psm", [1, TC], F32)
            for dt in range(DT):
                mm(mean_ps[:], ones128[:], rbs[dt][:],
                   start=(dt == 0), stop=(dt == DT - 1))
            ex2_ps = P("psm", [1, TC], F32)
            for dt in range(DT):
                mm(ex2_ps[:], ones128[:], sqs[dt][:],
                   start=(dt == 0), stop=(dt == DT - 1))
            meanS = pools["rows"].tile([1, TC], F32R, tag="rows")
            nc.vector.tensor_copy(meanS[:], mean_ps[:])
            msq = pools["rows"].tile([1, TC], F32R, tag="rows")
            nc.scalar.square(msq[:], meanS[:])
            var = pools["rows"].tile([1, TC], F32R, tag="rows")
            nc.vector.tensor_tensor(var[:], ex2_ps[:], msq[:], ALU.subtract)
            sstd = pools["rows"].tile([1, TC], F32R, tag="rows")
            nc.scalar.activation(sstd[:], var[:], AF.Sqrt, bias=eps_t[:])
            rstd = pools["rows"].tile([1, TC], F32R, tag="rows")
            nc.vector.reciprocal(rstd[:], sstd[:])
            meanB = P("psm", [128, TC], F32)
            mm(meanB[:], ones1r[:], meanS[:], start=True, stop=True)
            rstdB = P("psm", [128, TC], F32)
            mm(rstdB[:], ones1r[:], rstd[:], start=True, stop=True)
            xs, xbs = [], []
            for dt in range(DT):
                t1 = P("t1", [128, TC], F32)
                nc.vector.tensor_tensor(t1[:], rs[dt][:], meanB[:],
                                        ALU.subtract)
                t2 = P("t2", [128, TC], F32)
                nc.vector.tensor_tensor(t2[:], t1[:], rstdB[:], ALU.mult)
                xo = P("xres", [128, TC], F32)
                nc.vector.tensor_scalar(
                    out=xo[:], in0=t2[:],
                    scalar1=sm[:, _SM[gname] + dt: _SM[gname] + dt + 1],
                    scalar2=sm[:, _SM[bname] + dt: _SM[bname] + dt + 1],
                    op0=ALU.mult, op1=ALU.add)
                xob = P("xb", [128, TC], BF16)
                nc.vector.tensor_copy(xob[:], xo[:])
                xs.append(xo)
                xbs.append(xob)
            return xs, xbs

        def layer(l, x, xb):
            sm = P("smalls", [128, _SMW], F32)
            nc.sync.dma_start(sm[:], smalls[l])

            def col(name, i):
                c = _SM[name] + i
                return sm[:, c:c + 1]

            # ---- K^T (transposed keys), then AllGather ----
            kcs, vcs = [], []
            kv_in_k = pools["dram"].tile([D * TC], BF16, tag="kv_in_k")
            kv_all_k = pools["dram"].tile([NCORES * D * TC], BF16,
                                          tag="kv_all_k",
                                          addr_space="Shared")
            kin = kv_in_k[:].rearrange("(d p c) -> d p c", d=DT, p=128)
            for dt in range(DT):
                wkt = P("wk", [128, 1024], BF16)
                nc.sync.dma_start(wkt[:], wk[l, dt])
                ps = P("psm", [128, TC], F32)
                for kt in range(DT):
                    mm(ps[:], wkt[:, 128 * kt:128 * (kt + 1)], xb[kt][:],
                       start=(kt == 0), stop=(kt == DT - 1))
                kc = P("kc", [128, TC], BF16)
                nc.scalar.activation(kc[:], ps[:], AF.Identity,
                                     bias=col("bk", dt))
                nc.sync.dma_start(kin[dt], kc[:])
                kcs.append(kc)
            if "no_ag" in variant:
                nc.sync.dma_start(kv_all_k[0:D * TC], kv_in_k[:])
            else:
                nc.gpsimd.collective_compute(
                    "AllGather", ALU.bypass, replica_groups=rg,
                    ins=[kv_in_k.opt()], outs=[kv_all_k.opt()])

            # ---- V (token-major, ones column per head), then AllGather ----
            kv_in_v = pools["dram"].tile([TC * VW], BF16, tag="kv_in_v")
            kv_all_v = pools["dram"].tile([NCORES * TC * VW], BF16,
                                          tag="kv_all_v",
                                          addr_space="Shared")
            wv_sb = []
            for kt in range(DT):
                wvt = P("wv", [128, 1024], BF16)
                nc.sync.dma_start(wvt[:], wv[l, kt])
                wv_sb.append(wvt)
            vin = kv_in_v[:].rearrange("(t p c) -> t p c", t=4, p=128)
            for tt in range(4):
                vc = P("vc", [128, VW], BF16)
                vcr = vc[:].rearrange("p (h c) -> p h c", c=HD + 1)
                nc.vector.memset(vcr[:, :, HD:HD + 1], 1.0)
                for half in range(2):
                    ps = P("psm", [128, 512], F32)
                    for kt in range(DT):
                        mm(ps[:], xb[kt][:, 128 * tt:128 * (tt + 1)],
                           wv_sb[kt][:, 512 * half:512 * (half + 1)],
                           start=(kt == 0), stop=(kt == DT - 1))
                    nc.vector.tensor_copy(
                        vcr[:, 8 * half:8 * (half + 1), 0:HD],
                        ps[:].rearrange("p (h c) -> p h c", c=HD))
                nc.sync.dma_start(vin[tt], vc[:])
                vcs.append(vc)
            if "no_ag" in variant:
                nc.sync.dma_start(kv_all_v[0:TC * VW], kv_in_v[:])
            else:
                nc.gpsimd.collective_compute(
                    "AllGather", ALU.bypass, replica_groups=rg,
                    ins=[kv_in_v.opt()], outs=[kv_all_v.opt()])

            # ---- Q^T (stays local) ----
            qT = []
            for dt in range(DT):
                wqt = P("wq", [128, 1024], BF16)
                nc.sync.dma_start(wqt[:], wq[l, dt])
                ps = P("psm", [128, TC], F32)
                for kt in range(DT):
                    mm(ps[:], wqt[:, 128 * kt:128 * (kt + 1)], xb[kt][:],
                       start=(kt == 0), stop=(kt == DT - 1))
                qt = P("qT", [128, TC], BF16)
                nc.scalar.activation(qt[:], ps[:], AF.Identity,
                                     bias=col("bq", dt))
                qT.append(qt)

            # ---- attention ----
            kall = kv_all_k[:].rearrange("(s d p c) -> s d p c",
                                         s=NCORES, d=DT, p=128)
            vall = kv_all_v[:].rearrange("(s t c) -> s t c", s=NCORES, t=TC)
            ctxT = [P("ctxT", [128, TC], BF16) for _ in range(DT)]
            for hp in range(HP):
                for b in range(B):
                    if "local_kv" in variant:
                        kT = kcs[hp]
                    else:
                        kT = P("kT", [128, NCORES * TB], BF16)
                        src = kall[:, hp, :, TB * b:TB * (b + 1)].rearrange(
                            "s p c -> p s c")
                        nc.sync.dma_start(
                            kT[:].rearrange("p (s c) -> p s c", s=NCORES),
                            src)
                    exps = {}
                    for hsub in range(2):
                        for s in range(NCORES):
                            ps = P("pss", [128, 2 * TB], F32)
                            for k2 in range(2):
                                kcol = (128 * k2 if "local_kv" in variant
                                        else 256 * s + 128 * k2)
                                mm(ps[:, TB * k2:TB * (k2 + 1)],
                                   kT[64 * hsub:64 * (hsub + 1),
                                      kcol:kcol + 128],
                                   qT[hp][64 * hsub:64 * (hsub + 1),
                                          TB * b:TB * (b + 1)],
                                   start=True, stop=True)
                            ex = P("exp", [128, 2 * TB], BF16)
                            if "no_exp" in variant:
                                nc.vector.tensor_copy(ex[:], ps[:])
                            else:
                                nc.scalar.activation(ex[:], ps[:], AF.Exp,
                                                     scale=1.0 / np.sqrt(HD))
                            exps[(hsub, s)] = ex
                    ctx_ps = [P("psc", [HD + 1, TB], F32) for _ in range(2)]
                    if "local_kv" not in variant:
                        # all 16 key-tiles' V slices for this head pair in
                        # one 4D-AP DMA: sbuf col = s*260 + k2*130 + c
                        vsb = P("vs", [128, 16 * 130], BF16)
                        vsb4 = vsb[:].rearrange("p (s k c) -> p s k c",
                                                s=NCORES, k=2)
                        vall4 = vall[:, TB * b:TB * (b + 1),
                                     130 * hp:130 * (hp + 1)].rearrange(
                                         "s (k p) c -> p s k c", k=2)
                        for k2 in range(2):
                            nc.sync.dma_start(vsb4[:, :, k2],
                                              vall4[:, :, k2])
                    for kt in range(2 * NCORES):
                        s_, k2 = kt // 2, kt % 2
                        if "local_kv" in variant:
                            vs = vcs[kt % 4][:, 130 * hp:130 * (hp + 1)]
                        else:
                            vs = vsb[:, 260 * s_ + 130 * k2:
                                     260 * s_ + 130 * (k2 + 1)]
                        for hsub in range(2):
                            mm(ctx_ps[hsub][:],
                               vs[:, 65 * hsub:65 * (hsub + 1)],
                               exps[(hsub, s_)][:, TB * k2:TB * (k2 + 1)],
                               start=(kt == 0), stop=(kt == 2 * NCORES - 1))
                    for hsub in range(2):
                        rec = pools["rows"].tile([1, TB], F32R, tag="rows")
                        nc.vector.reciprocal(rec[:],
                                             ctx_ps[hsub][HD:HD + 1, :])
                        rB = P("psc", [HD, TB], F32)
                        mm(rB[:], ones1r[:, 0:HD], rec[:],
                           start=True, stop=True)
                        rBs = P("recb", [HD, TB], F32)
                        nc.scalar.activation(rBs[:], rB[:], AF.Copy)
                        t = P("ctxp", [HD, TB], F32)
                        nc.vector.tensor_tensor(t[:], ctx_ps[hsub][0:HD, :],
                                                rBs[:], ALU.mult)
                        nc.vector.tensor_scalar(
                            out=ctxT[hp][64 * hsub:64 * (hsub + 1),
                                         TB * b:TB * (b + 1)],
                            in0=t[:],
                            scalar1=sm[64 * hsub:64 * (hsub + 1),
                                       _SM["bv"] + hp:_SM["bv"] + hp + 1],
                            scalar2=None, op0=ALU.add)

            # ---- Wo + residual, then LN1 ----
            rs, rbs = [], []
            for dt in range(DT):
                wot = P("wo", [128, 1024], BF16)
                nc.sync.dma_start(wot[:], wo[l, dt])
                ps = P("psm", [128, TC], F32)
                for kt in range(DT):
                    mm(ps[:], wot[:, 128 * kt:128 * (kt + 1)], ctxT[kt][:],
                       start=(kt == 0), stop=(kt == DT - 1))
                r = P("xres", [128, TC], F32)
                nc.vector.scalar_tensor_tensor(
                    r[:], ps[:], col("bo", dt), x[dt][:], ALU.add, ALU.add)
                rb = P("rb", [128, TC], BF16)
                nc.vector.tensor_copy(rb[:], r[:])
                rs.append(r)
                rbs.append(rb)
            x2, x2b = layernorm(rs, rbs, sm, "g1", "be1")

            # ---- FFN ----
            h1 = []
            for ft in range(FT):
                w1t = P("w1", [128, 1024], BF16)
                nc.sync.dma_start(w1t[:], w1[l, ft])
                ps = P("psm", [128, TC], F32)
                for kt in range(DT):
                    mm(ps[:], w1t[:, 128 * kt:128 * (kt + 1)], x2b[kt][:],
                       start=(kt == 0), stop=(kt == DT - 1))
                h1t = P("h1", [128, TC], BF16)
                nc.scalar.activation(h1t[:], ps[:], AF.Relu,
                                     bias=col("b1", ft))
                h1.append(h1t)
            rs, rbs = [], []
            for dt in range(DT):
                ps = P("psm", [128, TC], F32)
                for q4 in range(4):
                    w2t = P("w2", [128, 1024], BF16)
                    nc.sync.dma_start(
                        w2t[:], w2[l, dt][:, 1024 * q4:1024 * (q4 + 1)])
                    for k8 in range(8):
                        kt = 8 * q4 + k8
                        mm(ps[:], w2t[:, 128 * k8:128 * (k8 + 1)],
                           h1[kt][:], start=(kt == 0), stop=(kt == FT - 1))
                r = P("xres", [128, TC], F32)
                nc.vector.scalar_tensor_tensor(
                    r[:], ps[:], col("b2", dt), x2[dt][:], ALU.add, ALU.add)
                rb = P("rb", [128, TC], BF16)
                nc.vector.tensor_copy(rb[:], r[:])
                rs.append(r)
                rbs.append(rb)
            return layernorm(rs, rbs, sm, "g2", "be2")

        # load x, run layers, store
        x, xb = [], []
        xv = xT32.rearrange("(d p) c -> d p c", p=128)
        xbv = xTbf.rearrange("(d p) c -> d p c", p=128)
        for dt in range(DT):
            xt = P("xres", [128, TC], F32)
            nc.sync.dma_start(xt[:], xv[dt])
            x.append(xt)
            xbt = P("xb", [128, TC], BF16)
            nc.sync.dma_start(xbt[:], xbv[dt])
            xb.append(xbt)
        for _ in range(reps):
            for l in range(L):
                x, xb = layer(l, x, xb)
        ov = outT.rearrange("(d p) c -> d p c", p=128)
        for dt in range(DT):
            nc.sync.dma_start(ov[dt], x[dt][:])

    nc.compile()
    return nc


def _pos_encoding():
    pos = np.arange(S, dtype=np.float32)[:, None]
    div = np.exp(np.arange(0, D, 2, dtype=np.float32)
                 * np.float32(-np.log(10000.0) / D))
    pe = np.zeros((S, D), dtype=np.float32)
    pe[:, 0::2] = np.sin(pos * div)
    pe[:, 1::2] = np.cos(pos * div)
    return pe


def _prep_inputs(tokens, mask, emb, Wq, bq, Wk, bk, Wv, bv, Wo, bo,
                 W1, b1, W2, b2, g1, be1, g2, be2):
    bf = ml_dtypes.bfloat16
    f32 = np.float32

    def np32(a):
        return np.ascontiguousarray(np.asarray(a, dtype=f32))

    x = np32(emb)[np.asarray(tokens)] + _pos_encoding()[None]  # [B, S, D]

    def colmajor(w, n_out_tiles):
        # w [L, K, N] -> [L, n_out_tiles, 128, K] with cols kt-major
        Lk, K, N = w.shape
        r = w.reshape(Lk, K // 128, 128, n_out_tiles, N // n_out_tiles)
        return np.ascontiguousarray(
            r.transpose(0, 3, 2, 1, 4).reshape(Lk, n_out_tiles, 128, -1)
        ).astype(bf)

    shared = {
        "wq": colmajor(np32(Wq), DT),
        "wk": colmajor(np32(Wk), DT),
        "wo": colmajor(np32(Wo), DT),
        "wv": np.ascontiguousarray(
            np32(Wv).reshape(L, DT, 128, D)).astype(bf),
        "w1": colmajor(np32(W1), FT),
        "w2": colmajor(np32(W2), DT),
    }
    smalls = np.zeros((L, 128, _SMW), dtype=f32)
    for name, vec, nt in [("bq", bq, DT), ("bk", bk, DT), ("bv", bv, DT),
                          ("bo", bo, DT), ("b2", b2, DT), ("g1", g1, DT),
                          ("be1", be1, DT), ("g2", g2, DT), ("be2", be2, DT),
                          ("b1", b1, FT)]:
        v = np32(vec).reshape(L, nt, 128)
        smalls[:, :, _SM[name]:_SM[name] + nt] = v.transpose(0, 2, 1)
    shared["smalls"] = smalls

    in_maps = []
    for c in range(NCORES):
        xc = np.concatenate(
            [x[0, TB * c:TB * (c + 1)], x[1, TB * c:TB * (c + 1)]], axis=0)
        xT = np.ascontiguousarray(xc.T.astype(f32))
        m = dict(shared)
        m["xT32"] = xT
        m["xTbf"] = xT.astype(bf)
        in_maps.append(m)
    return in_maps


_NC_CACHE = None


def _get_nc():
    global _NC_CACHE
    if _NC_CACHE is None:
        _NC_CACHE = _build_nc()
    return _NC_CACHE


def _unshard(results):
    out = np.empty((B, S, D), dtype=np.float32)
    for c in range(NCORES):
        xc = results[c]["outT"].T  # [TC, D]
        out[0, TB * c:TB * (c + 1)] = xc[:TB]
        out[1, TB * c:TB * (c + 1)] = xc[TB:]
    return out


def kernel(**inputs) -> np.ndarray:
    in_maps = _prep_inputs(**inputs)
    nc = _get_nc()
    res = bass_utils.run_bass_kernel_spmd(
        nc, in_maps, core_ids=list(range(NCORES)))
    return _unshard(res.results)


# revision 30
# speedup vs baseline: 29.2176x; 1.1403x over previous
"""Trainium2 Bass kernel for nn_Encoder_14121852469955.

6-layer post-LN transformer encoder (D=1024, H=16, F=4096, S=2048, B=2),
distributed over 8 NeuronCores.

Sharding: token-data-parallel. Each core owns 512 token rows (256 from each
batch element, an S/8 slice). Everything except attention is per-token and
runs locally with replicated weights. For attention, each layer AllGathers
K^T and V (bf16) across the 8 cores so every core attends its queries over
the full sequence of both batch elements.

On-device layout: activations are kept TRANSPOSED, xT = [D(partitions), T],
so every projection is a natural PE matmul (out = lhsT.T @ rhs with
contraction on partitions). Attention computes transposed scores
S^T = K @ Q^T per head, exp via the scalar engine (max-subtraction-free
softmax), and context via (V' appended with a ones column) so the softmax
denominator falls out of the same matmul chain. Matmul operands are bf16
(fp32 PSUM accumulation); the residual stream stays fp32 in SBUF.

Host side (inside kernel()): embedding gather + sinusoidal positions,
weight reshape/cast to bf16, sharding, and the final unshard/transpose.
"""

import numpy as np
import ml_dtypes

import concourse.mybir as mybir
import concourse.tile as tile
from concourse import bacc
from concourse import bass_utils

BF16 = mybir.dt.bfloat16
F32 = mybir.dt.float32
F32R = mybir.dt.float32r
AF = mybir.ActivationFunctionType
ALU = mybir.AluOpType

V, D, H, L, F, S, B = 32000, 1024, 16, 6, 4096, 2048, 2
HD = D // H  # 64
NCORES = 8
TB = S // NCORES  # 256 tokens per batch element per core
TC = B * TB  # 512 token rows per core
DT = D // 128  # 8 d-tiles
FT = F // 128  # 32 f-tiles
HP = H // 2  # 8 head pairs
VW = H * (HD + 1)  # 1040: V with a ones column interleaved per head
EPS = 1e-5

# column offsets inside the per-layer "smalls" [128, 104] tile
_SM = {"bq": 0, "bk": 8, "bv": 16, "bo": 24, "b2": 32,
       "g1": 40, "be1": 48, "g2": 56, "be2": 64, "b1": 72}
_SMW = 104


def _build_nc(reps=1, variant=()):
    """variant: experiment flags for timing A/B (results become garbage):
    "no_ag" (replace AllGathers with a local copy), "no_exp" (DVE copy
    instead of scalar-engine Exp), "local_kv" (attention reads local SBUF
    tiles instead of DMAing gathered K/V), "no_ln" (skip layernorm math),
    "no_norm_tail" (skip softmax-denominator normalization),
    "one_psum" (single shared psum pool)."""
    variant = set(variant)
    nc = bacc.Bacc("TRN2", target_bir_lowering=False, debug=False,
                   num_devices=NCORES)

    def inp(name, shape, dt):
        return nc.dram_tensor(name, shape, dt, kind="ExternalInput").ap()

    xT32 = inp("xT32", [D, TC], F32)
    xTbf = inp("xTbf", [D, TC], BF16)
    wq = inp("wq", [L, DT, 128, 1024], BF16)
    wk = inp("wk", [L, DT, 128, 1024], BF16)
    wo = inp("wo", [L, DT, 128, 1024], BF16)
    wv = inp("wv", [L, DT, 128, 1024], BF16)
    w1 = inp("w1", [L, FT, 128, 1024], BF16)
    w2 = inp("w2", [L, DT, 128, 4096], BF16)
    smalls = inp("smalls", [L, 128, _SMW], F32)
    outT = nc.dram_tensor("outT", [D, TC], F32, kind="ExternalOutput").ap()

    rg = [list(range(NCORES))]

    import contextlib
    with tile.TileContext(nc) as tc, contextlib.ExitStack() as ctx:
        ctx.enter_context(nc.allow_low_precision(
            "f32r norm scalars are intentional (~2^-13 rounding)"))
        pools = {}
        for name, bufs, space in [
            ("const", 1, "SBUF"), ("smalls", 2, "SBUF"),
            ("xres", 10, "SBUF"), ("xb", 10, "SBUF"),
            ("rb", 2, "SBUF"), ("sq", 2, "SBUF"),
            ("t1", 2, "SBUF"), ("t2", 2, "SBUF"),
            ("rows", 8, "SBUF"),
            ("kc", 9 if "local_kv" in variant else 3, "SBUF"),
            ("vc", 6 if "local_kv" in variant else 3, "SBUF"),
            ("kT", 2, "SBUF"), ("exp", 17, "SBUF"), ("vs", 3, "SBUF"),
            ("recb", 2, "SBUF"), ("ctxp", 2, "SBUF"),
            ("ctxT", 9, "SBUF"), ("qT", 9, "SBUF"),
            ("wq", 2, "SBUF"), ("wk", 2, "SBUF"), ("wo", 2, "SBUF"),
            ("wv", 9, "SBUF"), ("w1", 4, "SBUF"), ("w2", 4, "SBUF"),
            ("h1", 33, "SBUF"),
            ("pss", 2, "PSUM"), ("psc", 3, "PSUM"), ("psm", 3, "PSUM"),
            ("dram", 2, "DRAM"),
        ]:
            if name in ("pss", "psc", "psm"):
                continue
            pools[name] = ctx.enter_context(
                tc.tile_pool(name=name, bufs=bufs, space=space))
        pool_ps = ctx.enter_context(
            tc.tile_pool(name="ps", bufs=8, space="PSUM"))
        pools["pss"] = pools["psc"] = pools["psm"] = pool_ps

        def P(name, shape, dt):
            if name in ("pss", "psc", "psm"):
                return pools[name].tile(shape, dt, tag="ps", name=name)
            return pools[name].tile(shape, dt, tag=name, name=name)

        # constants
        ones128 = P("const", [128, 1], BF16)
        nc.vector.memset(ones128[:], 1.0 / D)
        ones1f = pools["const"].tile([1, 128], F32, tag="ones1f")
        nc.vector.memset(ones1f[:], 1.0)
        ones1r = pools["const"].tile([1, 128], F32R, tag="ones1r")
        nc.vector.tensor_copy(ones1r[:], ones1f[:])
        eps_t = pools["const"].tile([1, 1], F32, tag="eps_t")
        nc.vector.memset(eps_t[:], EPS)

        mm = nc.tensor.matmul

        def layernorm(rs, rbs, sm, gname, bname):
            """post-LN over d (partitions): rs fp32 [128,TC]x8, rbs bf16."""
            if "no_ln" in variant:
                xbs = []
                for dt in range(DT):
                    xob = P("xb", [128, TC], BF16)
                    nc.vector.tensor_copy(xob[:], rs[dt][:])
                    xbs.append(xob)
                return rs, xbs
            sqs = []
            for dt in range(DT):
                sqt = P("sq", [128, TC], BF16)
                nc.scalar.square(sqt[:], rbs[dt][:])
                sqs.append(sqt)
            mean_ps = P("psm", [1, TC], F32)
            for dt in range(DT):
                mm(mean_ps[:], ones128[:], rbs[dt][:],
                   start=(dt == 0), stop=(dt == DT - 1))
            ex2_ps = P("psm", [1, TC], F32)
            for dt in range(DT):
                mm(ex2_ps[:], ones128[:], sqs[dt][:],
                   start=(dt == 0), stop=(dt == DT - 1))
            meanS = pools["rows"].tile([1, TC], F32R, tag="rows")
            nc.vector.tensor_copy(meanS[:], mean_ps[:])
            msq = pools["rows"].tile([1, TC], F32R, tag="rows")
            nc.scalar.square(msq[:], meanS[:])
            var = pools["rows"].tile([1, TC], F32R, tag="rows")
            nc.vector.tensor_tensor(var[:], ex2_ps[:], msq[:], ALU.subtract)
            sstd = pools["rows"].tile([1, TC], F32R, tag="rows")
            nc.scalar.activation(sstd[:], var[:], AF.Sqrt, bias=eps_t[:])
            rstd = pools["rows"].tile([1, TC], F32R, tag="rows")
            nc.vector.reciprocal(rstd[:], sstd[:])
            meanB = P("psm", [128, TC], F32)
            mm(meanB[:], ones1r[:], meanS[:], start=True, stop=True)
            rstdB = P("psm", [128, TC], F32)
            mm(rstdB[:], ones1r[:], rstd[:], start=True, stop=True)
            xs, xbs = [], []
            for dt in range(DT):
                t1 = P("t1", [128, TC], F32)
                nc.vector.tensor_tensor(t1[:], rs[dt][:], meanB[:],
                                        ALU.subtract)
                t2 = P("t2", [128, TC], F32)
                nc.vector.tensor_tensor(t2[:], t1[:], rstdB[:], ALU.mult)
                xo = P("xres", [128, TC], F32)
                nc.vector.tensor_scalar(
                    out=xo[:], in0=t2[:],
                    scalar1=sm[:, _SM[gname] + dt: _SM[gname] + dt + 1],
                    scalar2=sm[:, _SM[bname] + dt: _SM[bname] + dt + 1],
                    op0=ALU.mult, op1=ALU.add)
                xob = P("xb", [128, TC], BF16)
                nc.vector.tensor_copy(xob[:], xo[:])
                xs.append(xo)
                xbs.append(xob)
            return xs, xbs

        def layer(l, x, xb):
            sm = P("smalls", [128, _SMW], F32)
            nc.sync.dma_start(sm[:], smalls[l])

            def col(name, i):
                c = _SM[name] + i
                return sm[:, c:c + 1]

            # ---- K^T (transposed keys), then AllGather ----
            kcs, vcs = [], []
            kv_in_k = pools["dram"].tile([D * TC], BF16, tag="kv_in_k")
            kv_all_k = pools["dram"].tile([NCORES * D * TC], BF16,
                                          tag="kv_all_k",
                                          addr_space="Shared")
            kin = kv_in_k[:].rearrange("(d p c) -> d p c", d=DT, p=128)
            for dt in range(DT):
                wkt = P("wk", [128, 1024], BF16)
                nc.sync.dma_start(wkt[:], wk[l, dt])
                ps = P("psm", [128, TC], F32)
                for kt in range(DT):
                    mm(ps[:], wkt[:, 128 * kt:128 * (kt + 1)], xb[kt][:],
                       start=(kt == 0), stop=(kt == DT - 1))
                kc = P("kc", [128, TC], BF16)
                nc.scalar.activation(kc[:], ps[:], AF.Identity,
                                     bias=col("bk", dt))
                nc.sync.dma_start(kin[dt], kc[:])
                kcs.append(kc)
            if "no_ag" in variant:
                nc.sync.dma_start(kv_all_k[0:D * TC], kv_in_k[:])
            else:
                nc.gpsimd.collective_compute(
                    "AllGather", ALU.bypass, replica_groups=rg,
                    ins=[kv_in_k.opt()], outs=[kv_all_k.opt()])

            # ---- V (token-major, ones column per head), then AllGather ----
            kv_in_v = pools["dram"].tile([TC * VW], BF16, tag="kv_in_v")
            kv_all_v = pools["dram"].tile([NCORES * TC * VW], BF16,
                                          tag="kv_all_v",
                                          addr_space="Shared")
            wv_sb = []
            for kt in range(DT):
                wvt = P("wv", [128, 1024], BF16)
                nc.sync.dma_start(wvt[:], wv[l, kt])
                wv_sb.append(wvt)
            vin = kv_in_v[:].rearrange("(t p c) -> t p c", t=4, p=128)
            for tt in range(4):
                vc = P("vc", [128, VW], BF16)
                vcr = vc[:].rearrange("p (h c) -> p h c", c=HD + 1)
                nc.vector.memset(vcr[:, :, HD:HD + 1], 1.0)
                for half in range(2):
                    ps = P("psm", [128, 512], F32)
                    for kt in range(DT):
                        mm(ps[:], xb[kt][:, 128 * tt:128 * (tt + 1)],
                           wv_sb[kt][:, 512 * half:512 * (half + 1)],
                           start=(kt == 0), stop=(kt == DT - 1))
                    nc.vector.tensor_copy(
                        vcr[:, 8 * half:8 * (half + 1), 0:HD],
                        ps[:].rearrange("p (h c) -> p h c", c=HD))
                nc.sync.dma_start(vin[tt], vc[:])
                vcs.append(vc)
            if "no_ag" in variant:
                nc.sync.dma_start(kv_all_v[0:TC * VW], kv_in_v[:])
            else:
                nc.gpsimd.collective_compute(
                    "AllGather", ALU.bypass, replica_groups=rg,
                    ins=[kv_in_v.opt()], outs=[kv_all_v.opt()])

            # ---- Q^T (stays local) ----
            qT = []
            for dt in range(DT):
                wqt = P("wq", [128, 1024], BF16)
                nc.sync.dma_start(wqt[:], wq[l, dt])
                ps = P("psm", [128, TC], F32)
                for kt in range(DT):
                    mm(ps[:], wqt[:, 128 * kt:128 * (kt + 1)], xb[kt][:],
                       start=(kt == 0), stop=(kt == DT - 1))
                qt = P("qT", [128, TC], BF16)
                nc.scalar.activation(qt[:], ps[:], AF.Identity,
                                     bias=col("bq", dt))
                qT.append(qt)

            # ---- attention ----
            kall = kv_all_k[:].rearrange("(s d p c) -> s d p c",
                                         s=NCORES, d=DT, p=128)
            vall = kv_all_v[:].rearrange("(s t c) -> s t c", s=NCORES, t=TC)
            ctxT = [P("ctxT", [128, TC], BF16) for _ in range(DT)]
            for hp in range(HP):
                for b in range(B):
                    if "local_kv" in variant:
                        kT = kcs[hp]
                    else:
                        kT = P("kT", [128, NCORES * TB], BF16)
                        src = kall[:, hp, :, TB * b:TB * (b + 1)].rearrange(
                            "s p c -> p s c")
                        nc.sync.dma_start(
                            kT[:].rearrange("p (s c) -> p s c", s=NCORES),
                            src)
                    exps = {}
                    for hsub in range(2):
                        for s in range(NCORES):
                            ps = P("pss", [128, 2 * TB], F32)
                            for k2 in range(2):
                                kcol = (128 * k2 if "local_kv" in variant
                                        else 256 * s + 128 * k2)
                                mm(ps[:, TB * k2:TB * (k2 + 1)],
                                   kT[64 * hsub:64 * (hsub + 1),
                                      kcol:kcol + 128],
                                   qT[hp][64 * hsub:64 * (hsub + 1),
                                          TB * b:TB * (b + 1)],
                                   start=True, stop=True)
                            ex = P("exp", [128, 2 * TB], BF16)
                            if "no_exp" in variant:
                                nc.vector.tensor_copy(ex[:], ps[:])
                            else:
                                nc.scalar.activation(ex[:], ps[:], AF.Exp,
                                                     scale=1.0 / np.sqrt(HD))
                            exps[(hsub, s)] = ex
                    ctx_ps = [P("psc", [HD + 1, TB], F32) for _ in range(2)]
                    if "local_kv" not in variant:
                        # all 16 key-tiles' V slices for this head pair in
                        # one 4D-AP DMA: sbuf col = s*260 + k2*130 + c
                        vsb = P("vs", [128, 16 * 130], BF16)
                        vsb4 = vsb[:].rearrange("p (s k c) -> p s k c",
                                                s=NCORES, k=2)
                        vall4 = vall[:, TB * b:TB * (b + 1),
                                     130 * hp:130 * (hp + 1)].rearrange(
                                         "s (k p) c -> p s k c", k=2)
                        for k2 in range(2):
                            nc.sync.dma_start(vsb4[:, :, k2],
                                              vall4[:, :, k2])
                    for kt in range(2 * NCORES):
                        s_, k2 = kt // 2, kt % 2
                        if "local_kv" in variant:
                            vs = vcs[kt % 4][:, 130 * hp:130 * (hp + 1)]
                        else:
                            vs = vsb[:, 260 * s_ + 130 * k2:
                                     260 * s_ + 130 * (k2 + 1)]
                        for hsub in range(2):
                            mm(ctx_ps[hsub][:],
                               vs[:, 65 * hsub:65 * (hsub + 1)],
                               exps[(hsub, s_)][:, TB * k2:TB * (k2 + 1)],
                               start=(kt == 0), stop=(kt == 2 * NCORES - 1))
                    for hsub in range(2):
                        if "no_norm_tail" in variant:
                            nc.scalar.activation(
                                ctxT[hp][64 * hsub:64 * (hsub + 1),
                                         TB * b:TB * (b + 1)],
                                ctx_ps[hsub][0:HD, :], AF.Copy)
                            continue
                        rec = pools["rows"].tile([1, TB], F32R, tag="rows")
                        nc.vector.reciprocal(rec[:],
                                             ctx_ps[hsub][HD:HD + 1, :])
                        rB = P("psc", [HD, TB], F32)
                        mm(rB[:], ones1r[:, 0:HD], rec[:],
                           start=True, stop=True)
                        rBs = P("recb", [HD, TB], F32)
                        nc.scalar.activation(rBs[:], rB[:], AF.Copy)
                        # bv is folded into bo on the host (bo' = bo+bv@Wo),
                        # so the normalized context writes out directly.
                        nc.vector.tensor_tensor(
                            ctxT[hp][64 * hsub:64 * (hsub + 1),
                                     TB * b:TB * (b + 1)],
                            ctx_ps[hsub][0:HD, :], rBs[:], ALU.mult)

            # ---- Wo + residual, then LN1 ----
            rs, rbs = [], []
            for dt in range(DT):
                wot = P("wo", [128, 1024], BF16)
                nc.sync.dma_start(wot[:], wo[l, dt])
                ps = P("psm", [128, TC], F32)
                for kt in range(DT):
                    mm(ps[:], wot[:, 128 * kt:128 * (kt + 1)], ctxT[kt][:],
                       start=(kt == 0), stop=(kt == DT - 1))
                r = P("xres", [128, TC], F32)
                nc.vector.scalar_tensor_tensor(
                    r[:], ps[:], col("bo", dt), x[dt][:], ALU.add, ALU.add)
                rb = P("rb", [128, TC], BF16)
                nc.vector.tensor_copy(rb[:], r[:])
                rs.append(r)
                rbs.append(rb)
            x2, x2b = layernorm(rs, rbs, sm, "g1", "be1")

            # ---- FFN ----
            h1 = []
            for ft in range(FT):
                w1t = P("w1", [128, 1024], BF16)
                nc.sync.dma_start(w1t[:], w1[l, ft])
                ps = P("psm", [128, TC], F32)
                for kt in range(DT):
                    mm(ps[:], w1t[:, 128 * kt:128 * (kt + 1)], x2b[kt][:],
                       start=(kt == 0), stop=(kt == DT - 1))
                h1t = P("h1", [128, TC], BF16)
                nc.scalar.activation(h1t[:], ps[:], AF.Relu,
                                     bias=col("b1", ft))
                h1.append(h1t)
            rs, rbs = [], []
            for dt in range(DT):
                ps = P("psm", [128, TC], F32)
                for q4 in range(4):
                    w2t = P("w2", [128, 1024], BF16)
                    nc.sync.dma_start(
                        w2t[:], w2[l, dt][:, 1024 * q4:1024 * (q4 + 1)])
                    for k8 in range(8):
                        kt = 8 * q4 + k8
                        mm(ps[:], w2t[:, 128 * k8:128 * (k8 + 1)],
                           h1[kt][:], start=(kt == 0), stop=(kt == FT - 1))
                r = P("xres", [128, TC], F32)
                nc.vector.scalar_tensor_tensor(
                    r[:], ps[:], col("b2", dt), x2[dt][:], ALU.add, ALU.add)
                rb = P("rb", [128, TC], BF16)
                nc.vector.tensor_copy(rb[:], r[:])
                rs.append(r)
                rbs.append(rb)
            return layernorm(rs, rbs, sm, "g2", "be2")

        # load x, run layers, store
        x, xb = [], []
        xv = xT32.rearrange("(d p) c -> d p c", p=128)
        xbv = xTbf.rearrange("(d p) c -> d p c", p=128)
        for dt in range(DT):
            xt = P("xres", [128, TC], F32)
            nc.sync.dma_start(xt[:], xv[dt])
            x.append(xt)
            xbt = P("xb", [128, TC], BF16)
            nc.sync.dma_start(xbt[:], xbv[dt])
            xb.append(xbt)
        for _ in range(reps):
            for l in range(L):
                x, xb = layer(l, x, xb)
        ov = outT.rearrange("(d p) c -> d p c", p=128)
        for dt in range(DT):
            nc.sync.dma_start(ov[dt], x[dt][:])

    nc.compile()
    return nc


def _pos_encoding():
    pos = np.arange(S, dtype=np.float32)[:, None]
    div = np.exp(np.arange(0, D, 2, dtype=np.float32)
                 * np.float32(-np.log(10000.0) / D))
    pe = np.zeros((S, D), dtype=np.float32)
    pe[:, 0::2] = np.sin(pos * div)
    pe[:, 1::2] = np.cos(pos * div)
    return pe


def _prep_inputs(tokens, mask, emb, Wq, bq, Wk, bk, Wv, bv, Wo, bo,
                 W1, b1, W2, b2, g1, be1, g2, be2):
    bf = ml_dtypes.bfloat16
    f32 = np.float32

    def np32(a):
        return np.ascontiguousarray(np.asarray(a, dtype=f32))

    x = np32(emb)[np.asarray(tokens)] + _pos_encoding()[None]  # [B, S, D]

    def colmajor(w, n_out_tiles):
        # w [L, K, N] -> [L, n_out_tiles, 128, K] with cols kt-major
        Lk, K, N = w.shape
        r = w.reshape(Lk, K // 128, 128, n_out_tiles, N // n_out_tiles)
        return np.ascontiguousarray(
            r.transpose(0, 3, 2, 1, 4).reshape(Lk, n_out_tiles, 128, -1)
        ).astype(bf)

    shared = {
        "wq": colmajor(np32(Wq), DT),
        "wk": colmajor(np32(Wk), DT),
        "wo": colmajor(np32(Wo), DT),
        "wv": np.ascontiguousarray(
            np32(Wv).reshape(L, DT, 128, D)).astype(bf),
        "w1": colmajor(np32(W1), FT),
        "w2": colmajor(np32(W2), DT),
    }
    bo_eff = np32(bo) + np.einsum("ld,ldo->lo", np32(bv), np32(Wo))
    smalls = np.zeros((L, 128, _SMW), dtype=f32)
    for name, vec, nt in [("bq", bq, DT), ("bk", bk, DT), ("bv", bv, DT),
                          ("bo", bo_eff, DT), ("b2", b2, DT), ("g1", g1, DT),
                          ("be1", be1, DT), ("g2", g2, DT), ("be2", be2, DT),
                          ("b1", b1, FT)]:
        v = np32(vec).reshape(L, nt, 128)
        smalls[:, :, _SM[name]:_SM[name] + nt] = v.transpose(0, 2, 1)
    shared["smalls"] = smalls

    in_maps = []
    for c in range(NCORES):
        xc = np.concatenate(
            [x[0, TB * c:TB * (c + 1)], x[1, TB * c:TB * (c + 1)]], axis=0)
        xT = np.ascontiguousarray(xc.T.astype(f32))
        m = dict(shared)
        m["xT32"] = xT
        m["xTbf"] = xT.astype(bf)
        in_maps.append(m)
    return in_maps


_NC_CACHE = None


def _get_nc():
    global _NC_CACHE
    if _NC_CACHE is None:
        _NC_CACHE = _build_nc()
    return _NC_CACHE


def _unshard(results):
    out = np.empty((B, S, D), dtype=np.float32)
    for c in range(NCORES):
        xc = results[c]["outT"].T  # [TC, D]
        out[0, TB * c:TB * (c + 1)] = xc[:TB]
        out[1, TB * c:TB * (c + 1)] = xc[TB:]
    return out


def kernel(**inputs) -> np.ndarray:
    in_maps = _prep_inputs(**inputs)
    nc = _get_nc()
    res = bass_utils.run_bass_kernel_spmd(
        nc, in_maps, core_ids=list(range(NCORES)))
    return _unshard(res.results)
